# revision 3
# baseline (speedup 1.0000x reference)
"""Causal attention head (B=8, C=2048, E=1024, H=64) with post-softmax query-row
zero mask, on 8 TRN2 NeuronCores — data-parallel over batch (one batch per core).

Per-core dataflow (all matmuls bf16 -> f32 PSUM):
  qkT = [Wq|Wk]^T-style packed projection: psum [128, 512] per 512-col chunk of x^T
        (rows 0:64 = q^T, rows 64:128 = k^T; split into q_sb / k_sb via cross-base copies)
  vT  = Wv projection [64, 2048], then PE-transposed into v_aug tiles [128j, 65]
        whose column 64 is 1.0 (so the AV matmul also produces softmax row-sums).
  scoresT[j, q] tiles = k_sb-chunk (stationary) @ q_sb (moving); exp via ScalarE with
        the C**-0.5 scale fused in; causal diagonal handled by restricting the moving
        range + one triangular-mask multiply per diagonal tile.
  outT[65, q] = v_aug (stationary) @ p (moving), accumulated over j-chunks; row 64
        = softmax denominators. Normalize + zero-mask on device; host transposes back.
"""

import numpy as np
import ml_dtypes

import concourse.bass as bass
import concourse.bacc as bacc
import concourse.mybir as mybir
import concourse.tile as tile
from concourse.bass_utils import run_bass_kernel_spmd
from concourse.masks import make_identity

B, C, E, H = 8, 2048, 1024, 64
EC = E // 128          # 8 contraction chunks
QC = C // 512          # 4 query chunks of 512
NJ = C // 128          # 16 key chunks of 128
SCALE = float(C) ** -0.5
BF16 = mybir.dt.bfloat16
F32 = mybir.dt.float32

_CACHED = {}


def _build():
    nc = bacc.Bacc("TRN2", target_bir_lowering=False, debug=False, num_devices=B)
    xt_ext = nc.dram_tensor("xt", [E, C], BF16, kind="ExternalInput")
    wqk_ext = nc.dram_tensor("wqk", [E, 2 * H], BF16, kind="ExternalInput")
    wv_ext = nc.dram_tensor("wv", [E, H], BF16, kind="ExternalInput")
    tri_ext = nc.dram_tensor("tri", [128, 128], BF16, kind="ExternalInput")
    cm_ext = nc.dram_tensor("cmask", [1, C], F32, kind="ExternalInput")
    out_ext = nc.dram_tensor("out", [H, C], F32, kind="ExternalOutput")

    with tile.TileContext(nc) as tc:
        with (
            tc.tile_pool(name="const", bufs=1) as const_pool,
            tc.tile_pool(name="xt", bufs=1) as xt_pool,
            tc.tile_pool(name="acts", bufs=1) as act_pool,
            tc.tile_pool(name="p", bufs=3) as p_pool,
            tc.tile_pool(name="osb", bufs=2) as o_pool,
            tc.tile_pool(name="bc", bufs=2) as bc_pool,
            tc.tile_pool(name="mm", bufs=3, space="PSUM") as mm_pool,
            tc.tile_pool(name="po", bufs=2, space="PSUM") as po_pool,
            tc.tile_pool(name="tr", bufs=2, space="PSUM") as tr_pool,
        ):
            # constants
            wqk_sb = const_pool.tile([128, EC * 128], BF16)
            wv_sb = const_pool.tile([128, EC * H], BF16)
            tri_sb = const_pool.tile([128, 128], BF16)
            cm_row = const_pool.tile([1, C], F32)
            ident = const_pool.tile([128, 128], BF16)
            for e in range(EC):
                nc.sync.dma_start(wqk_sb[:, e * 128:(e + 1) * 128],
                                  wqk_ext.ap()[e * 128:(e + 1) * 128, :])
                nc.sync.dma_start(wv_sb[:, e * H:(e + 1) * H],
                                  wv_ext.ap()[e * 128:(e + 1) * 128, :])
            nc.sync.dma_start(tri_sb[:], tri_ext.ap())
            nc.sync.dma_start(cm_row[:], cm_ext.ap())
            make_identity(nc, ident[:])
            cm_bc = const_pool.tile([64, C], F32)
            nc.gpsimd.partition_broadcast(cm_bc[:], cm_row[:])

            xt_sb = xt_pool.tile([128, EC * C], BF16)
            q_sb = act_pool.tile([64, C], BF16)
            k_sb = act_pool.tile([64, C], BF16)
            vt_sb = act_pool.tile([64, C], BF16)
            vaug_sb = act_pool.tile([128, NJ * (H + 1)], BF16)
            nc.vector.memset(vaug_sb[:], 1.0)

            for c in range(QC):
                csl = slice(c * 512, (c + 1) * 512)
                # stream in x^T columns for this chunk, all e-rows
                for e in range(EC):
                    nc.sync.dma_start(
                        xt_sb[:, e * C + c * 512: e * C + (c + 1) * 512],
                        xt_ext.ap()[e * 128:(e + 1) * 128, csl])
                # QK projection (packed): psum rows 0:64 = q^T, 64:128 = k^T
                pq = mm_pool.tile([128, 512], F32, tag="mm")
                for e in range(EC):
                    nc.tensor.matmul(
                        pq[:], wqk_sb[:, e * 128:(e + 1) * 128],
                        xt_sb[:, e * C + c * 512: e * C + (c + 1) * 512],
                        start=(e == 0), stop=(e == EC - 1))
                nc.scalar.copy(q_sb[:, csl], pq[0:64, :])
                nc.scalar.copy(k_sb[:, csl], pq[64:128, :])
                # V projection
                pv = mm_pool.tile([64, 512], F32, tag="mm")
                for e in range(EC):
                    nc.tensor.matmul(
                        pv[:], wv_sb[:, e * H:(e + 1) * H],
                        xt_sb[:, e * C + c * 512: e * C + (c + 1) * 512],
                        start=(e == 0), stop=(e == EC - 1))
                nc.vector.tensor_copy(vt_sb[:, csl], pv[:])
                # transpose v^T chunks into v_aug tiles [128j, 65] (col 64 = 1.0)
                for jj in range(4):
                    jc = 4 * c + jj
                    pt = tr_pool.tile([128, H], BF16, tag="tr")
                    nc.tensor.transpose(
                        pt[:], vt_sb[:, jc * 128:(jc + 1) * 128],
                        ident[0:64, 0:64])
                    nc.vector.tensor_copy(
                        vaug_sb[:, jc * (H + 1): jc * (H + 1) + H], pt[:])

                # attention for query chunk qc == c
                qc = c
                po_t = po_pool.tile([H + 1, 512], F32, tag="po")
                nj = 4 * qc + 4
                for jc in range(nj):
                    r = jc - 4 * qc  # >= 0 on causal-diagonal tiles
                    qoff = 128 * r if r >= 0 else 0
                    ps = mm_pool.tile([128, 512], F32, tag="mm")
                    nc.tensor.matmul(
                        ps[:, qoff:512],
                        k_sb[:, jc * 128:(jc + 1) * 128],
                        q_sb[:, qc * 512 + qoff:(qc + 1) * 512],
                        start=True, stop=True)
                    p_t = p_pool.tile([128, 512], BF16, tag="p")
                    nc.scalar.activation(
                        p_t[:, qoff:512], ps[:, qoff:512],
                        mybir.ActivationFunctionType.Exp, scale=SCALE)
                    if r >= 0:
                        nc.vector.tensor_mul(
                            p_t[:, qoff:qoff + 128],
                            p_t[:, qoff:qoff + 128], tri_sb[:])
                    nc.tensor.matmul(
                        po_t[:, qoff:512],
                        vaug_sb[:, jc * (H + 1):(jc + 1) * (H + 1)],
                        p_t[:, qoff:512],
                        start=(jc == 0), stop=(jc == nj - 1))
                # normalize: out = outT[0:64] * cmask / sums  (sums = row 64)
                rec_row = bc_pool.tile([1, 512], F32, tag="rrow")
                nc.vector.reciprocal(rec_row[:], po_t[64:65, :])
                rec = bc_pool.tile([64, 512], F32, tag="rec")
                nc.gpsimd.partition_broadcast(rec[:], rec_row[:])
                nc.vector.tensor_mul(rec[:], rec[:], cm_bc[:, qc * 512:(qc + 1) * 512])
                o_t = o_pool.tile([64, 512], F32, tag="o")
                nc.vector.tensor_mul(o_t[:], po_t[0:64, :], rec[:])
                nc.sync.dma_start(out_ext.ap()[:, qc * 512:(qc + 1) * 512], o_t[:])

    nc.compile()
    return nc


def kernel(x, Wq, Wk, Wv, zero_mask):
    x = np.asarray(x)
    if "nc" not in _CACHED:
        _CACHED["nc"] = _build()
    nc = _CACHED["nc"]

    bf = ml_dtypes.bfloat16
    wqk = np.concatenate([np.asarray(Wq), np.asarray(Wk)], axis=1).astype(bf)
    wv = np.asarray(Wv).astype(bf)
    tri = np.triu(np.ones((128, 128), dtype=np.float32)).astype(bf)  # tri[j,q]=1 iff j<=q
    in_maps = []
    for b in range(B):
        in_maps.append({
            "xt": np.ascontiguousarray(x[b].T).astype(bf),
            "wqk": wqk,
            "wv": wv,
            "tri": tri,
            "cmask": (~np.asarray(zero_mask[b]))[None, :].astype(np.float32),
        })
    res = run_bass_kernel_spmd(nc, in_maps, core_ids=list(range(B)))
    out = np.empty((B, C, H), dtype=np.float32)
    for b in range(B):
        out[b] = res.results[b]["out"].T
    return out


# revision 7
# speedup vs baseline: 1.3621x; 1.3621x over previous
"""Causal attention head (B=8, C=2048, E=1024, H=64) with post-softmax query-row
zero mask, on 8 TRN2 NeuronCores — data-parallel over batch (one batch per core).

Per-core dataflow (all matmuls bf16 -> f32 PSUM):
  qk^T packed projection: psum [128, 512] per 512-col chunk of x^T
        (rows 0:64 = q^T, rows 64:128 = k^T; split into q_sb / k_sb)
  v^T  = Wv projection [64, 2048], PE-transposed into v_aug tiles [128j, 65]
        whose column 64 is 1.0 (the AV matmul then also emits softmax row-sums).
  scoresT[j, q] tiles = k-chunk (stationary) @ q (moving); exp on ScalarE with
        the C**-0.5 scale fused in; causal diagonal = restricted moving range
        + one triangular-mask multiply per diagonal tile.
  outT[65, q] = v_aug (stationary) @ p (moving), accumulated over j-chunks; row
        64 = softmax denominators. Normalized on device (fast reciprocal +
        broadcast); the query-row zero mask and the final transpose are applied
        host-side while unsharding.
"""

import numpy as np
import ml_dtypes

import concourse.bass as bass
import concourse.bacc as bacc
import concourse.mybir as mybir
import concourse.tile as tile
from concourse.bass_utils import run_bass_kernel_spmd
from concourse.masks import make_identity

B, C, E, H = 8, 2048, 1024, 64
EC = E // 128          # 8 contraction chunks
QC = C // 512          # 4 query chunks of 512
NJ = C // 128          # 16 key chunks of 128
SCALE = float(C) ** -0.5
BF16 = mybir.dt.bfloat16
F32 = mybir.dt.float32

_CACHED = {}


def _build():
    nc = bacc.Bacc("TRN2", target_bir_lowering=False, debug=False, num_devices=B)
    xt_ext = nc.dram_tensor("xt", [E, C], BF16, kind="ExternalInput")
    wqk_ext = nc.dram_tensor("wqk", [E, 2 * H], BF16, kind="ExternalInput")
    wv_ext = nc.dram_tensor("wv", [E, H], BF16, kind="ExternalInput")
    tri_ext = nc.dram_tensor("tri", [128, 128], BF16, kind="ExternalInput")
    out_ext = nc.dram_tensor("out", [H, C], F32, kind="ExternalOutput")

    with tile.TileContext(nc) as tc:
        with (
            tc.tile_pool(name="const", bufs=1) as const_pool,
            tc.tile_pool(name="acts", bufs=1) as act_pool,
            tc.tile_pool(name="p", bufs=3) as p_pool,
            tc.tile_pool(name="osb", bufs=2) as o_pool,
            tc.tile_pool(name="bc", bufs=2) as bc_pool,
            tc.tile_pool(name="mmp", bufs=2, space="PSUM") as mmp_pool,
            tc.tile_pool(name="mms", bufs=4, space="PSUM") as mms_pool,
            tc.tile_pool(name="po", bufs=2, space="PSUM") as po_pool,
        ):
            # ---- constants + all input DMAs, front-loaded ----
            wqk_sb = const_pool.tile([128, EC * 128], BF16)
            wv_sb = const_pool.tile([128, EC * H], BF16)
            tri_sb = const_pool.tile([128, 128], BF16)
            ident = const_pool.tile([128, 128], BF16)
            for e in range(EC):
                nc.sync.dma_start(wqk_sb[:, e * 128:(e + 1) * 128],
                                  wqk_ext.ap()[e * 128:(e + 1) * 128, :])
                nc.sync.dma_start(wv_sb[:, e * H:(e + 1) * H],
                                  wv_ext.ap()[e * 128:(e + 1) * 128, :])
            nc.sync.dma_start(tri_sb[:], tri_ext.ap())
            make_identity(nc, ident[:])

            xt_sb = act_pool.tile([128, EC * C], BF16)
            for c in range(QC):
                for e in range(EC):
                    nc.sync.dma_start(
                        xt_sb[:, e * C + c * 512: e * C + (c + 1) * 512],
                        xt_ext.ap()[e * 128:(e + 1) * 128, c * 512:(c + 1) * 512])

            q_sb = act_pool.tile([64, C], BF16)
            k_sb = act_pool.tile([64, C], BF16)
            vt_sb = act_pool.tile([64, C], BF16)
            vaug_sb = act_pool.tile([128, NJ * (H + 1)], BF16)
            nc.vector.memset(vaug_sb[:], 1.0)

            for c in range(QC):
                csl = slice(c * 512, (c + 1) * 512)
                # QK projection (packed): psum rows 0:64 = q^T, 64:128 = k^T
                pq = mmp_pool.tile([128, 512], F32, tag="mm")
                for e in range(EC):
                    nc.tensor.matmul(
                        pq[:], wqk_sb[:, e * 128:(e + 1) * 128],
                        xt_sb[:, e * C + c * 512: e * C + (c + 1) * 512],
                        start=(e == 0), stop=(e == EC - 1))
                nc.vector.tensor_copy(q_sb[:, csl], pq[0:64, :])
                nc.vector.tensor_copy(k_sb[:, csl], pq[64:128, :])
                # V projection
                pv = mmp_pool.tile([64, 512], F32, tag="mm")
                for e in range(EC):
                    nc.tensor.matmul(
                        pv[:], wv_sb[:, e * H:(e + 1) * H],
                        xt_sb[:, e * C + c * 512: e * C + (c + 1) * 512],
                        start=(e == 0), stop=(e == EC - 1))
                nc.vector.tensor_copy(vt_sb[:, csl], pv[:])
                # transpose v^T chunks into v_aug tiles [128j, 65] (col 64 = 1.0)
                for jj in range(4):
                    jc = 4 * c + jj
                    pt = mmp_pool.tile([128, H], BF16, tag="mm")
                    nc.tensor.transpose(
                        pt[:], vt_sb[:, jc * 128:(jc + 1) * 128],
                        ident[0:64, 0:64])
                    nc.vector.tensor_copy(
                        vaug_sb[:, jc * (H + 1): jc * (H + 1) + H], pt[:])

                # ---- attention for query chunk qc == c ----
                qc = c
                po_t = po_pool.tile([H + 1, 512], F32, tag="po")
                nj = 4 * qc + 4
                for jc in range(nj):
                    r = jc - 4 * qc  # >= 0 on causal-diagonal tiles
                    qoff = 128 * r if r >= 0 else 0
                    ps = mms_pool.tile([128, 512], F32, tag="mms")
                    nc.tensor.matmul(
                        ps[:, qoff:512],
                        k_sb[:, jc * 128:(jc + 1) * 128],
                        q_sb[:, qc * 512 + qoff:(qc + 1) * 512],
                        start=True, stop=True)
                    p_t = p_pool.tile([128, 512], BF16, tag="p")
                    nc.scalar.activation(
                        p_t[:, qoff:512], ps[:, qoff:512],
                        mybir.ActivationFunctionType.Exp, scale=SCALE)
                    if r >= 0:
                        nc.vector.tensor_mul(
                            p_t[:, qoff:qoff + 128],
                            p_t[:, qoff:qoff + 128], tri_sb[:])
                    nc.tensor.matmul(
                        po_t[:, qoff:512],
                        vaug_sb[:, jc * (H + 1):(jc + 1) * (H + 1)],
                        p_t[:, qoff:512],
                        start=(jc == 0), stop=(jc == nj - 1))
                # normalize: out = outT[0:64] / sums (sums = row 64)
                sum_row = bc_pool.tile([1, 512], F32, tag="srow")
                nc.vector.tensor_copy(sum_row[:], po_t[64:65, :])
                rec_row = bc_pool.tile([1, 512], F32, tag="rrow")
                nc.vector.reciprocal_approx_fast(rec_row[:], sum_row[:])
                rec = bc_pool.tile([64, 512], F32, tag="rec")
                nc.gpsimd.partition_broadcast(rec[:], rec_row[:])
                o_t = o_pool.tile([64, 512], F32, tag="o")
                nc.vector.tensor_mul(o_t[:], po_t[0:64, :], rec[:])
                nc.sync.dma_start(out_ext.ap()[:, qc * 512:(qc + 1) * 512], o_t[:])

    nc.compile()
    return nc


def make_in_maps(x, Wq, Wk, Wv, zero_mask):
    bf = ml_dtypes.bfloat16
    wqk = np.concatenate([np.asarray(Wq), np.asarray(Wk)], axis=1).astype(bf)
    wv = np.asarray(Wv).astype(bf)
    tri = np.triu(np.ones((128, 128), dtype=np.float32)).astype(bf)  # tri[j,q]=1 iff j<=q
    return [{
        "xt": np.ascontiguousarray(np.asarray(x[b]).T).astype(bf),
        "wqk": wqk,
        "wv": wv,
        "tri": tri,
    } for b in range(B)]


def kernel(x, Wq, Wk, Wv, zero_mask):
    if "nc" not in _CACHED:
        _CACHED["nc"] = _build()
    nc = _CACHED["nc"]
    in_maps = make_in_maps(x, Wq, Wk, Wv, zero_mask)
    res = run_bass_kernel_spmd(nc, in_maps, core_ids=list(range(B)))
    zm = np.asarray(zero_mask)
    out = np.empty((B, C, H), dtype=np.float32)
    for b in range(B):
        out[b] = res.results[b]["out"].T
        out[b][zm[b]] = 0.0
    return out
